# revision 1
# baseline (speedup 1.0000x reference)
"""2-layer GAT on 8 Trainium2 NeuronCores (Bass/Tile, SPMD).

Sharding: destination nodes i are partitioned across the 8 cores (512 rows
each); each core computes softmax + aggregation over all N=4096 sources for
its slice, both layers. The layer-1 projection g = x @ W1 is computed
replicated on every core (an AllGather of g would be slower than the 27µs of
PE time: measured collectives run ~50 GB/s with ~10µs latency). Layer 1 runs
as four single-head passes; each pass software-pipelines the projection
(PIPE=3 tiles ahead) into the attention j-loop so the in-order PE queue never
stalls on the ACT PSUM->SBUF copy of g, and the per-head PSUM budget
(4 accumulators + 2 projection slots + 2 misc) fits the 8 banks.

exp() is folded INTO the score custom-DVE op via the Schraudolph bit trick,
so each edge is touched exactly once, by one engine (DVE, the only engine
that can combine mask+score at 1 elem/lane/cycle), and the Activation engine
is freed for the PSUM copies + normalization. In bf16-bit space,
  bits(exp(lrelu(z))) ~= round(K*lrelu(z) + B),  K = 128/ln2, B = 127*128
and lrelu commutes with the affine map:
  y = max(zb, 0.2*zb + 0.8*B),  zb = K*z + B.
The DVE op computes relu(max(zb, zb*C2 + C1)) with zb = (K*s + (K*t + B)) +
maskneg and writes int16 (in-range, never saturates); the int16 tile is
bitcast to bf16 for the aggregation matmuls (lhsT = p chunks, rhs = [g | 1]
so Z rides along in the 257th column). Masked edges carry maskneg = -98304,
which drives both max() branches negative, so relu() clamps them to +0.0
exactly. The +0.5 in B makes truncation act as round-to-nearest; constant
bit offsets are per-row scales that cancel in softmax. Measured accuracy
cost of fastexp end-to-end: ~7e-3 rel (gate 2e-2).

Inputs are host-relaid so every SBUF tile loads with a handful of large
DMAs (the SP queue pays ~0.5µs dispatch per DMA): mask as [128, 32*512]
bf16 j-tile-major, t1k as [128, 32*4]. The score stream's operands (srep,
t1k, first mask chunks) are DMA'd first; xt/w1 ride the ACT HWDGE queue.
Between layers one [4096, 66] bf16 AllGather moves g2_aug = [g2 | 1 | t2];
the gathered tensor reloads as a single rearranged DMA and t2 is read
through a strided view. The final divide-by-Z and transpose happen on host
from the returned [CLS+1, 512] raw slices.
"""

import numpy as np
import ml_dtypes

import concourse.bass as bass
import concourse.bacc as bacc
import concourse.mybir as mybir
import concourse.tile as tile
from concourse.bass_utils import run_bass_kernel_spmd
from concourse.masks import make_identity

N = 4096
IN = 256
HID = 256
HEADS = 4
CLS = 64
SLOPE = 0.2
NCORES = 8
IS = N // NCORES          # 512 destination rows per core
ICHUNKS = IS // 128       # 4
JT = N // 128             # 32 source-node tiles

KEXP = 128.0 / np.log(2.0)          # bf16-bit fastexp slope
BEXP = 127.0 * 128.0 + 0.5          # exponent bias + round-to-nearest bias
MASKNEG = -98304.0                  # bf16-exact; forces both lrelu branches < 0

F32 = mybir.dt.float32
BF16 = mybir.dt.bfloat16
I16 = mybir.dt.int16
ADD = mybir.AluOpType.add
MULT = mybir.AluOpType.mult
AF = mybir.ActivationFunctionType

BF = ml_dtypes.bfloat16

# ---- custom fused DVE op: p_bits = relu(max(zb, zb*C2 + C1)), zb=in0+s0+in1
import concourse.dve_ops as _dve_ops
from concourse.dve_spec import Spec as _Spec, Src0 as _Src0, Src1 as _Src1, \
    C0 as _C0, C1 as _C1, C2 as _C2, Zero as _Zero, maxx as _maxx, \
    lower as _dve_lower, _has_src1
from concourse.dve_uop import DveOpSpec as _DveOpSpec


def _gat_p_ref(in0, in1, s0, s1, imm2):
    zb = (in0.astype(np.float32) + s0) + in1.astype(np.float32)
    y = np.maximum(zb, zb * imm2 + s1)
    return np.maximum(y, 0.0)


def _register(name, spec):
    if name in _dve_ops._SUB_OPCODE_FOR_NAME:
        return next(o for o in _dve_ops.OPS if o.name == name)
    opcode = _dve_ops._CUSTOM_DVE_ROW_BASE + len(_dve_ops.OPS)
    assert opcode < 0x20
    shas = {}
    for ver in ("v3", "v4"):
        s = _DveOpSpec(name=name, opcode=opcode,
                       uops=_dve_lower(spec, ver=ver), rd1_en=_has_src1(spec))
        shas[ver] = s.sha(ver)
    op = _dve_ops.DveOp(name, spec, subdim=False, uops_sha=shas)
    _dve_ops.OPS.append(op)
    _dve_ops._SUB_OPCODE_FOR_NAME[name] = opcode
    _dve_ops.CUSTOM_DVE_SPECS[name] = spec
    return op


_zb = (_Src0 + _C0) + _Src1
GAT_P = _register("GAT_P",
                  _Spec(body=_maxx(_maxx(_zb, _zb * _C2 + _C1), _Zero),
                        reference=_gat_p_ref))

_NC_CACHE = None


def build(reps=1, collectives=True):
    nc = bacc.Bacc("TRN2", target_bir_lowering=False, debug=False,
                   num_devices=NCORES if collectives else 1)

    xt = nc.dram_tensor("xt", [IN, N], BF16, kind="ExternalInput")
    w1 = nc.dram_tensor("w1", [IN, HEADS * HID], BF16, kind="ExternalInput")
    srep = nc.dram_tensor("srep", [128, HEADS * IS], BF16, kind="ExternalInput")
    t1k = nc.dram_tensor("t1k", [128, JT * HEADS], F32, kind="ExternalInput")
    maskt = nc.dram_tensor("maskt", [128, JT * IS], BF16, kind="ExternalInput")
    w2a = nc.dram_tensor("w2a", [HID, CLS + 2], BF16, kind="ExternalInput")
    y = nc.dram_tensor("y", [CLS + 1, IS], F32, kind="ExternalOutput")

    gath2 = [nc.dram_tensor(f"gath2_{r}", [N, CLS + 2], BF16,
                            kind="Internal", addr_space="Shared") for r in range(reps)]

    groups = [list(range(NCORES))]
    C1IMM = 0.8 * BEXP

    with tile.TileContext(nc) as tc:
        with (
            tc.tile_pool(name="sb", bufs=1) as sb,        # persistent tiles
            tc.tile_pool(name="wk", bufs=3) as wk,        # rotating work tiles
            tc.tile_pool(name="ps", bufs=8, space="PSUM") as ps,
            tc.tile_pool(name="dram", bufs=1, space="DRAM") as dram,
        ):
            # ---- resident inputs -------------------------------------------------
            ident = sb.tile([128, 128], BF16, tag="ident", name="ident")
            make_identity(nc, ident[:])

            # DMA order matters: the DVE score stream needs srep/t1k/mask
            # first (SP queue); the projection operands xt/w1 go on the ACT
            # HWDGE queue so their dispatch doesn't delay the mask chunks.
            srep_sb = sb.tile([128, HEADS * IS], BF16, tag="srep", name="sreps")
            nc.sync.dma_start(srep_sb[:, 0:IS], srep[:, 0:IS])
            t1k_sb = sb.tile([128, JT * HEADS], F32, tag="t1k", name="t1ks")
            nc.sync.dma_start(t1k_sb[:], t1k[:, :])
            mask_all = sb.tile([128, JT * IS], BF16, tag="mka", name="mka")
            mchunks = [4, 4, 8, 8, 8]
            moff = 0
            for mc in mchunks[:2]:
                nc.sync.dma_start(mask_all[:, moff * IS:(moff + mc) * IS],
                                  maskt[:, moff * IS:(moff + mc) * IS])
                moff += mc
            nc.sync.dma_start(srep_sb[:, IS:], srep[:, IS:])
            for mc in mchunks[2:]:
                nc.sync.dma_start(mask_all[:, moff * IS:(moff + mc) * IS],
                                  maskt[:, moff * IS:(moff + mc) * IS])
                moff += mc
            mask_sb = [mask_all[:, j * IS:(j + 1) * IS] for j in range(JT)]
            xt_sb = [sb.tile([128, N], BF16, tag=f"xt{k}", name=f"xt{k}") for k in range(2)]
            w1_sb = [sb.tile([128, HEADS * HID], BF16, tag=f"w1{k}", name=f"w1s{k}") for k in range(2)]
            w2a_sb = [sb.tile([128, CLS + 2], BF16, tag=f"w2a{k}", name=f"w2as{k}") for k in range(2)]
            for k in range(2):
                nc.scalar.dma_start(xt_sb[k][:], xt[k * 128:(k + 1) * 128, :])
                nc.scalar.dma_start(w1_sb[k][:], w1[k * 128:(k + 1) * 128, :])
                nc.scalar.dma_start(w2a_sb[k][:], w2a[k * 128:(k + 1) * 128, :])

            for rep in range(reps):
                # ---- layer-1: two head-pair passes; the projection half needed
                # by each pass is computed inside it (PE), g tiles rotate.
                PIPE = 3

                def emit_proj(h, j):
                    # projection for head h, j-tile j: g = [g_h | 1]
                    g = wk.tile([128, HID + 1], BF16, tag="g", name="g", bufs=PIPE + 3)
                    pj = ps.tile([128, HID], F32, tag="pj", name="pj", bufs=2)
                    for k in range(2):
                        nc.tensor.matmul(
                            pj[:],
                            lhsT=xt_sb[k][:, j * 128:(j + 1) * 128],
                            rhs=w1_sb[k][:, h * HID:(h + 1) * HID],
                            start=(k == 0), stop=(k == 1),
                        )
                    nc.scalar.copy(g[:, 0:HID], pj[:])
                    nc.gpsimd.memset(g[:, HID:HID + 1], 1.0)
                    return g

                contrib = {}
                for h in range(HEADS):
                    agg = {}
                    for m in range(ICHUNKS):
                        agg[m] = ps.tile([128, HID + 1], F32, tag="agps",
                                         name=f"agg{h}_{m}", bufs=4)
                    gq = [emit_proj(h, j) for j in range(PIPE)]
                    for jn in range(JT):
                        j = jn
                        # fused score+exp: p bits, int16 (+0.0 on masked edges)
                        p = wk.tile([128, IS], I16, tag="p", name="p", bufs=14)
                        nc.vector._custom_dve(
                            GAT_P,
                            out=p[:],
                            in0=srep_sb[:, h * IS:(h + 1) * IS],
                            in1=mask_sb[j],
                            s0=t1k_sb[:, j * HEADS + h:j * HEADS + h + 1],
                            s1=C1IMM,
                            imm2=SLOPE,
                        )
                        if jn + PIPE < JT:
                            gq.append(emit_proj(h, jn + PIPE))
                        g = gq[jn]
                        for m in range(ICHUNKS):
                            nc.tensor.matmul(
                                agg[m][:],
                                lhsT=p[:, m * 128:(m + 1) * 128].bitcast(BF16),
                                rhs=g[:],
                                start=(jn == 0), stop=(jn == JT - 1),
                            )
                    # normalize: contrib = agg / Z  (head-mean scaling folded later)
                    for m in range(ICHUNKS):
                        rz = wk.tile([128, 1], F32, tag="rz", name="rz")
                        nc.vector.reciprocal(rz[:], agg[m][:, HID:HID + 1])
                        ct = sb.tile([128, HID], F32, tag=f"ct{h}_{m}", name=f"ct{h}_{m}")
                        nc.scalar.activation(ct[:], agg[m][:, 0:HID],
                                             AF.Copy, bias=0.0, scale=rz[:])
                        contrib[h, m] = ct

                # ---- head mean + ELU + g2_aug, in two halves; each half's
                # [N/2, 66] AllGather fires as soon as its two m-chunks are
                # done, so gather A overlaps the second ELU half and layer-2's
                # first 16 j-tiles overlap gather B.
                bounce2 = dram.tile([IS, CLS + 2], BF16, tag="b2", name="b2")
                ag2all = sb.tile([128, ICHUNKS * (CLS + 2)], BF16, tag="ag2a", name="ag2a")
                ht_sb = [sb.tile([128, IS], BF16, tag=f"ht{k}", name=f"ht{k}") for k in range(2)]
                s2own = sb.tile([128, ICHUNKS], F32, tag="s2own", name="s2own")

                def emit_chunk(m):
                    a0 = wk.tile([128, HID], BF16, tag="ha", name="ha")
                    nc.vector.tensor_tensor(a0[:], contrib[0, m][:], contrib[1, m][:], ADD)
                    a1 = wk.tile([128, HID], BF16, tag="hb", name="hb")
                    nc.vector.tensor_tensor(a1[:], contrib[2, m][:], contrib[3, m][:], ADD)
                    hm = wk.tile([128, HID], F32, tag="hm", name="hm")
                    nc.vector.tensor_tensor(hm[:], a0[:], a1[:], ADD)
                    # ELU on hm/4: r = relu(hm/4); u = exp(hm/4 - r); helu = (r-1)+u
                    r = wk.tile([128, HID], F32, tag="hr", name="hr")
                    nc.scalar.activation(r[:], hm[:], AF.Relu, bias=0.0, scale=0.25)
                    mn = wk.tile([128, HID], F32, tag="hn", name="hn")
                    nc.vector.scalar_tensor_tensor(
                        out=mn[:], in0=hm[:], scalar=0.25, in1=r[:],
                        op0=MULT, op1=mybir.AluOpType.subtract)
                    u = wk.tile([128, HID], F32, tag="hu", name="hu")
                    nc.scalar.activation(u[:], mn[:], AF.Exp)
                    helu = wk.tile([128, HID], BF16, tag="helu", name="helu")
                    nc.vector.scalar_tensor_tensor(
                        out=helu[:], in0=r[:], scalar=-1.0, in1=u[:], op0=ADD, op1=ADD)
                    for k in range(2):
                        pt = ps.tile([128, 128], BF16, tag="psm", name="pt", bufs=1)
                        nc.tensor.transpose(pt[:], helu[:, k * 128:(k + 1) * 128], ident[:])
                        nc.scalar.copy(ht_sb[k][:, m * 128:(m + 1) * 128], pt[:])
                    pg = ps.tile([128, CLS + 2], F32, tag="psm", name="pg", bufs=1)
                    for k in range(2):
                        nc.tensor.matmul(
                            pg[:], lhsT=ht_sb[k][:, m * 128:(m + 1) * 128],
                            rhs=w2a_sb[k][:], start=(k == 0), stop=(k == 1),
                        )
                    off = m * (CLS + 2)
                    nc.vector.tensor_copy(ag2all[:, off:off + CLS], pg[:, 0:CLS])
                    nc.vector.memset(ag2all[:, off + CLS:off + CLS + 1], 1.0)
                    nc.vector.tensor_copy(ag2all[:, off + CLS + 1:off + CLS + 2], pg[:, CLS:CLS + 1])
                    nc.vector.tensor_copy(s2own[:, m:m + 1], pg[:, CLS + 1:CLS + 2])

                HC = CLS + 2
                for m in range(ICHUNKS):
                    emit_chunk(m)
                nc.sync.dma_start(
                    bounce2[:].rearrange("(a b) c -> b a c", b=128),
                    ag2all[:].rearrange("p (a c) -> p a c", c=HC))
                if collectives:
                    nc.gpsimd.collective_compute(
                        "AllGather", mybir.AluOpType.bypass, replica_groups=groups,
                        ins=[bounce2[:, :]], outs=[gath2[rep][:, :]],
                    )
                else:
                    nc.gpsimd.dma_start(gath2[rep][0:IS, :], bounce2[:, :])

                # one rearranged reload of the gathered g2_aug [N, 66]
                g2all = sb.tile([128, JT * HC], BF16, tag="g2a", name="g2a")
                nc.sync.dma_start(
                    g2all[:].rearrange("p (a c) -> p a c", c=HC),
                    gath2[rep].rearrange("(a b) c -> b a c", b=128))
                t2view = g2all[:].rearrange("p (a c) -> p a c", c=HC)[:, :, CLS + 1:CLS + 2]
                t2k_sb = sb.tile([128, JT], F32, tag="t2k", name="t2k")
                nc.vector.tensor_scalar(out=t2k_sb[:], in0=t2view,
                                        scalar1=float(KEXP), scalar2=float(BEXP),
                                        op0=MULT, op1=ADD)

                # ---- s2 broadcast: [512] column -> [128, 512] rows, scaled by K ----
                s2bf = wk.tile([128, ICHUNKS], BF16, tag="s2bf", name="s2bf")
                nc.vector.tensor_copy(s2bf[:], s2own[:])
                pt2 = ps.tile([1, IS], BF16, tag="big1", name="pt2", bufs=1)
                for m in range(ICHUNKS):
                    nc.tensor.transpose(
                        pt2[0:1, m * 128:(m + 1) * 128], s2bf[:, m:m + 1], ident[:])
                s2t = sb.tile([1, IS], BF16, tag="s2t", name="s2t")
                nc.vector.tensor_copy(s2t[:], pt2[:])
                onesk = sb.tile([1, 128], BF16, tag="onesk", name="onesk")
                nc.vector.memset(onesk[:], float(KEXP))
                pr = ps.tile([128, IS], F32, tag="big1", name="pr", bufs=1)
                nc.tensor.matmul(pr[:], lhsT=onesk[:], rhs=s2t[:], start=True, stop=True)
                s2rep = sb.tile([128, IS], BF16, tag="s2rep", name="s2rep")
                nc.vector.tensor_copy(s2rep[:], pr[:])

                # ---- layer-2 attention ----------------------------------------------
                agg2t = ps.tile([CLS + 1, 512], F32, tag="big1", name="agg2t", bufs=1)
                for jp in range(JT // 2):
                    p2 = wk.tile([128, 2 * IS], I16, tag="p2", name="p2", bufs=12)
                    for d in range(2):
                        j = 2 * jp + d
                        nc.vector._custom_dve(
                            GAT_P, out=p2[:, d * IS:(d + 1) * IS], in0=s2rep[:],
                            in1=mask_sb[j], s0=t2k_sb[:, j:j + 1],
                            s1=C1IMM, imm2=SLOPE)
                    for d in range(2):
                        j = 2 * jp + d
                        nc.tensor.matmul(
                            agg2t[:], lhsT=g2all[:, j * HC:j * HC + CLS + 1],
                            rhs=p2[:, d * IS:(d + 1) * IS].bitcast(BF16),
                            start=(j == 0), stop=(j == JT - 1),
                        )
                yt_sb = wk.tile([CLS + 1, 512], F32, tag="yt", name="yt")
                nc.vector.tensor_copy(yt_sb[:], agg2t[:])
                nc.sync.dma_start(y[:, :], yt_sb[:])

    nc.compile()
    return nc


def _get_nc():
    global _NC_CACHE
    if _NC_CACHE is None:
        _NC_CACHE = build()
    return _NC_CACHE


def kernel(x, adj_mat, W1, a1_src, a1_dst, W2, a2_src, a2_dst):
    x = np.asarray(x, dtype=np.float32)
    adj = np.asarray(adj_mat, dtype=bool)
    W1 = np.asarray(W1, dtype=np.float32)
    a1_src = np.asarray(a1_src, dtype=np.float32)
    a1_dst = np.asarray(a1_dst, dtype=np.float32)
    W2 = np.asarray(W2, dtype=np.float32)
    a2_src = np.asarray(a2_src, dtype=np.float32)
    a2_dst = np.asarray(a2_dst, dtype=np.float32)

    # host-side tiny precomputation (O(N*IN) matmuls with 8-col outputs)
    W1r = W1.astype(np.float64).reshape(IN, HEADS, HID)
    w1s = np.einsum("khf,f->kh", W1r, a1_src.astype(np.float64))
    w1d = np.einsum("khf,f->kh", W1r, a1_dst.astype(np.float64))
    xd = x.astype(np.float64)
    s1 = (xd @ w1s).astype(np.float32)          # [N, HEADS]
    t1 = (xd @ w1d).astype(np.float32)          # [N, HEADS]
    t1k = (t1 * np.float32(KEXP) + np.float32(BEXP)).astype(np.float32)
    t1k_r = np.ascontiguousarray(
        t1k.reshape(JT, 128, HEADS).transpose(1, 0, 2).reshape(128, JT * HEADS))
    w2aug = np.concatenate(
        [W2, (W2.astype(np.float64) @ a2_dst.astype(np.float64))[:, None].astype(np.float32),
         (W2.astype(np.float64) @ a2_src.astype(np.float64))[:, None].astype(np.float32)],
        axis=1,
    )                                            # [HID, CLS+2]: g2 | t2 | s2
    mask_neg = np.where((~adj).T, np.float32(MASKNEG), np.float32(0.0))  # [N(j), N(i)]
    xt_all = np.ascontiguousarray(x.T).astype(BF)                  # [IN, N]
    w1_bf = W1.astype(BF)
    w2a_bf = w2aug.astype(BF)
    s1k = (s1 * np.float32(KEXP)).astype(np.float32)

    in_maps = []
    for c in range(NCORES):
        isl = slice(c * IS, (c + 1) * IS)
        srep_c = np.broadcast_to(
            np.ascontiguousarray(s1k[isl].T).reshape(1, HEADS * IS), (128, HEADS * IS)
        ).astype(BF)
        mask_c = mask_neg[:, isl].reshape(JT, 128, IS).transpose(1, 0, 2)
        in_maps.append({
            "xt": xt_all,
            "w1": w1_bf,
            "srep": np.ascontiguousarray(srep_c),
            "t1k": t1k_r,
            "maskt": np.ascontiguousarray(mask_c.reshape(128, JT * IS)).astype(BF),
            "w2a": w2a_bf,
        })

    global _last_in_maps
    _last_in_maps = in_maps
    nc = _get_nc()
    res = run_bass_kernel_spmd(nc, in_maps, core_ids=list(range(NCORES)))
    outs = []
    for c in range(NCORES):
        raw = res.results[c]["y"]        # [CLS+1, IS]: rows 0:CLS unnorm, row CLS = Z
        outs.append((raw[0:CLS] / raw[CLS:CLS + 1]).T)
    return np.concatenate(outs, axis=0).astype(np.float32)

